# revision 24
# baseline (speedup 1.0000x reference)
"""
Trainium2 Bass kernel for EnhancedIsotropyMaximizationLoss.

loss = 1/diag_var_mean + log(mean(pairwise_L2_distance(c)))
where c = row-L2-normalized embeddings, centered by the column mean.

Key algebra:
  * pairwise distances are translation invariant, so the centering is NOT
    applied on device -- distances use the normalized (uncentered) rows e,
    and the variance term is corrected on host:
      sum(c*c) = sum(e*e) - N*||mu||^2,  mu = mean(e, axis=0).
  * per-row s_i = ||e_i||^2 = (||x_i|| / (||x_i|| + eps))^2 comes straight
    from the row-space norms (no transposed reduction needed).

Distribution (8 cores, no collectives): circulant decomposition of the
64x64 grid of 128-row tiles; core k gets the input rotated by k*1024 rows
and computes, for each local row-tile m in 0..7: diag tile (w=1),
d=1..31 (w=2), d=32 (w=1).  Column tiles beyond m+32 <= 39 are never
touched, so only the first 40 row-tiles (5 MB of 8 MB) are loaded,
normalized and transposed per core.  Partial sums combined on host.

On-device pipeline per core (engine-balanced, single PSUM lifetime so
phase A and the distance blocks overlap freely):
  DMA      x[5120,256] f32 in 10 transfers
  DVE      fused square+reduce row norms, diag(r) scale tiles (bf16 4x),
           sq2sum ct0 half, -2 lhsT copies, esum
  GpSimd   PSUM->SBUF drains of the transposed data, sq2sum ct1 half
  ScalarE  all distance sqrt+accumulate passes
  PE       transpose-with-scale matmuls (x_tile^T @ diag(r) -> e^T bf16),
           gram blocks (-2 e_i.e_j), +s_j ones-pass
"""

import sys

if "/opt/trn_rl_repo" not in sys.path:
    sys.path.insert(0, "/opt/trn_rl_repo")

import numpy as np

N, D, P = 8192, 256, 128
NT = N // P          # 64 row tiles
NCORES = 8
LT = NT // NCORES    # 8 local row tiles per core
NTU = LT + 32        # 40 row tiles actually used per core
NU = NTU * P         # 5120 rows loaded per core
EPS = 1e-6
# positivity guards under the sqrt; bf16 storage of sq2sum vs f32 PSUM gram
# can mismatch the diagonal by ~0.5% of s (~0.01), so the diag guard must
# cover that.  Off-diagonal sq_dists are >= ~1.3 for N(0,1) data.
DELTA_BG = 2e-2
DELTA_SM = 1e-4

_CACHE = {}


def _build(stage=5, reps=1):
    import concourse.bacc as bacc
    import concourse.tile as tile
    from concourse import masks, mybir

    Op = mybir.AluOpType
    Act = mybir.ActivationFunctionType
    F32 = mybir.dt.float32
    F32R = mybir.dt.float32r
    BF16 = mybir.dt.bfloat16
    AX = mybir.AxisListType

    nc = bacc.Bacc("TRN2", target_bir_lowering=False, debug=False)
    x = nc.dram_tensor("x", [NU, D], F32, kind="ExternalInput").ap()
    out = nc.dram_tensor("out", [P, 8], F32, kind="ExternalOutput").ap()

    with tile.TileContext(nc) as tc:
        from contextlib import ExitStack

        ctx = ExitStack()
        with ctx:
            singles = ctx.enter_context(tc.tile_pool(name="singles", bufs=1))
            ct = singles.tile([P, 2, NU], BF16, tag="ct")
            sq2 = singles.tile([P, NU], BF16, tag="sq2")
            sqt = singles.tile([P, NU], BF16, tag="sqt")
            lhsm2 = singles.tile([P, 3, 2, LT * P], BF16, tag="lhsm2")
            onesb = singles.tile([P, 3, P], BF16, tag="onesb")
            identb = singles.tile([P, P], BF16, tag="identb")
            s_loc = singles.tile([P, LT], F32, tag="s_loc")
            bias_sm = singles.tile([P, LT], F32, tag="bias_sm")
            bias_bg = singles.tile([P, LT], F32, tag="bias_bg")
            acc_w1 = singles.tile([P, LT], F32, tag="acc_w1")
            acc_w2 = singles.tile([P, 4 * LT], F32, tag="acc_w2")
            esum16 = singles.tile([P, 2, 8], F32, tag="esum16")
            esum = singles.tile([P, 2], F32, tag="esum")
            out_sb = singles.tile([P, 8], F32, tag="out_sb")

            onescol = singles.tile([P, 1], BF16, tag="onescol")
            nc.vector.memset(onescol[:], 1.0)
            for cp in range(3):
                nc.vector.tensor_copy(onesb[:, cp, :],
                                      onescol[:].to_broadcast([P, P]))
            masks.make_identity(nc, identb[:])
            if stage != 5:
                for tl in (s_loc, bias_sm, bias_bg, acc_w1, acc_w2):
                    nc.vector.memset(tl[:], 0.5)
                nc.vector.memset(esum[:], 0.0)
                nc.vector.memset(esum16[:], 0.0)

            for _rep in range(reps):
                GROUPS = [(0, 16), (16, 16), (32, 8)]
                with (
                    tc.tile_pool(name="xa", bufs=9) as xa_pool,
                    tc.tile_pool(name="ssg", bufs=3) as ssg_pool,
                    tc.tile_pool(name="dgr", bufs=8) as dg_pool,
                    tc.tile_pool(name="ptp", bufs=2, space="PSUM") as pt_pool,
                    tc.tile_pool(name="blk", bufs=2, space="PSUM") as blk_pool,
                    tc.tile_pool(name="trs", bufs=3) as tr_pool,
                    tc.tile_pool(name="trd", bufs=2) as trd_pool,
                ):
                    ct0v = ct[:, 0, :].rearrange("p (t c) -> p t c", c=P)
                    ct1v = ct[:, 1, :].rearrange("p (t c) -> p t c", c=P)
                    sq2v = sq2[:].rearrange("p (t c) -> p t c", c=P)
                    wrot = [0]

                    def norms(gt0, gn):
                        """DMA a group of row tiles + row norms -> r, ||e||^2."""
                        nrm_g = ssg_pool.tile([P, 16], F32, tag="nrm")
                        den_g = ssg_pool.tile([P, 16], F32, tag="den")
                        r_g = ssg_pool.tile([P, 16], F32, tag="rg")
                        sv_g = ssg_pool.tile([P, 16], F32, tag="sv")
                        bng = ssg_pool.tile([P, 16, 6], F32, tag="bng")
                        ta_g = ssg_pool.tile([P, 16], F32, tag="ta")
                        tb_g = ssg_pool.tile([P, 16], F32, tag="tb")
                        xbs = []
                        for qq in range(gn // 4):
                            t0 = gt0 + qq * 4
                            xb = xa_pool.tile([P, 4, D], F32, tag="xt")
                            dma_eng = nc.sync if qq % 2 == 0 else nc.scalar
                            dma_eng.dma_start(
                                out=xb[:],
                                in_=x[t0 * P:(t0 + 4) * P, :].rearrange(
                                    "(a p) d -> p a d", p=P))
                            # DVE one-pass stats per tile (row sumsq)
                            for j in range(4):
                                q = qq * 4 + j
                                nc.vector.bn_stats(bng[:, q, :], xb[:, j, :])
                            xbs.append(xb)
                        gsl = (slice(None), slice(0, gn))
                        # sumsq = M2_e + M2_o + 128*(mean_e^2 + mean_o^2)
                        nc.vector.tensor_tensor(
                            out=ta_g[gsl], in0=bng[:, 0:gn, 1],
                            in1=bng[:, 0:gn, 1], op=Op.mult)
                        nc.vector.tensor_tensor(
                            out=tb_g[gsl], in0=bng[:, 0:gn, 4],
                            in1=bng[:, 0:gn, 4], op=Op.mult)
                        nc.vector.tensor_tensor(
                            out=ta_g[gsl], in0=ta_g[gsl], in1=tb_g[gsl],
                            op=Op.add)
                        nc.vector.tensor_scalar(
                            out=ta_g[gsl], in0=ta_g[gsl], scalar1=float(D // 2),
                            scalar2=None, op0=Op.mult)
                        nc.vector.tensor_tensor(
                            out=nrm_g[gsl], in0=bng[:, 0:gn, 2],
                            in1=bng[:, 0:gn, 5], op=Op.add)
                        nc.vector.tensor_tensor(
                            out=nrm_g[gsl], in0=nrm_g[gsl], in1=ta_g[gsl],
                            op=Op.add)
                        # nrm = ||x||, r = 1/(nrm+EPS), s = (nrm*r)^2
                        nc.scalar.activation(nrm_g[gsl], nrm_g[gsl], Act.Sqrt)
                        nc.vector.tensor_scalar(
                            out=den_g[gsl], in0=nrm_g[gsl], scalar1=EPS,
                            scalar2=None, op0=Op.add)
                        nc.vector.reciprocal(r_g[gsl], den_g[gsl])
                        if gt0 == 0:
                            # s_i for the local tiles (bias terms)
                            lsl = (slice(None), slice(0, LT))
                            nc.vector.tensor_tensor(
                                out=sv_g[lsl], in0=nrm_g[lsl], in1=r_g[lsl],
                                op=Op.mult)
                            nc.vector.tensor_tensor(
                                out=s_loc[:], in0=sv_g[lsl], in1=sv_g[lsl],
                                op=Op.mult)
                            nc.vector.tensor_scalar(
                                out=bias_sm[:], in0=s_loc[:],
                                scalar1=DELTA_SM, scalar2=None, op0=Op.add)
                            nc.vector.tensor_scalar(
                                out=bias_bg[:], in0=s_loc[:],
                                scalar1=DELTA_BG, scalar2=None, op0=Op.add)
                        return xbs, r_g

                    def transposes(gt0, gn, xbs, r_g, upto=3):
                        """row scale -> bf16, PE transposes, DVE drains."""
                        for q in range(gn):
                            t = gt0 + q
                            q4 = q % 4
                            xbq = xbs[q // 4]
                            et = dg_pool.tile([P, D], BF16, tag="et")
                            # e = x * r, converted to bf16 (GpSimd; it has
                            # no PSUM port so the drains live on DVE)
                            nc.gpsimd.tensor_scalar(
                                out=et[:], in0=xbq[:, q4, :],
                                scalar1=r_g[:, q:q + 1], scalar2=None,
                                op0=Op.mult)
                            if upto < 2:
                                continue
                            if q4 == 0:
                                pt0 = pt_pool.tile([P, 4, P], BF16, tag="pt0")
                                pt1 = pt_pool.tile([P, 4, P], BF16, tag="pt1")
                            nc.tensor.transpose(
                                pt0[:, q4, :], et[:, 0:P], identb[:])
                            nc.tensor.transpose(
                                pt1[:, q4, :], et[:, P:2 * P], identb[:])
                            if q4 == 3 and upto >= 3:
                                c0 = (t - 3) * P
                                sl = slice(c0, c0 + 4 * P)
                                nc.vector.tensor_copy(
                                    ct[:, 0, sl],
                                    pt0[:].rearrange("p a d -> p (a d)"))
                                nc.vector.tensor_copy(
                                    ct[:, 1, sl],
                                    pt1[:].rearrange("p a d -> p (a d)"))

                    def lhs_esum():
                        """-2 lhsT copies + local e column sums."""
                        for cp in range(3):
                            for h in range(2):
                                nc.vector.tensor_scalar(
                                    out=lhsm2[:, cp, h, :],
                                    in0=ct[:, h, 0:LT * P],
                                    scalar1=-2.0, scalar2=None, op0=Op.mult)
                        nc.vector.tensor_reduce(
                            esum16[:],
                            ct[:, :, 0:LT * P].rearrange(
                                "p h (a c) -> p h a c", c=P),
                            axis=AX.X, op=Op.add)
                        nc.vector.tensor_reduce(
                            esum[:], esum16[:], axis=AX.X, op=Op.add)

                    def sq2sum(i0, i1):
                        """sq2 = ct0^2 + ct1^2 over 512-col chunks [i0, i1)."""
                        for i in range(i0, i1):
                            sl = slice(i * 512, (i + 1) * 512)
                            nc.vector.tensor_tensor(
                                out=sq2[:, sl], in0=ct[:, 0, sl],
                                in1=ct[:, 0, sl], op=Op.mult)
                            nc.gpsimd.tensor_tensor(
                                out=sqt[:, sl], in0=ct[:, 1, sl],
                                in1=ct[:, 1, sl], op=Op.mult)
                            nc.vector.tensor_tensor(
                                out=sq2[:, sl], in0=sq2[:, sl],
                                in1=sqt[:, sl], op=Op.add)

                    def lhs(mm, h):
                        return lhsm2[:, wrot[0] % 3, h, mm * P:(mm + 1) * P]

                    def ones_r():
                        return onesb[:, wrot[0] % 3, :]

                    # strip d=1..31 (3968 cols, w=2) per row-tile, in 4
                    # chunks of [1024, 1024, 1024, 896]; per-512 matmul
                    # sub-blocks, one wide sqrt+accum per chunk.
                    CHUNKS = [(0, 1024), (1024, 1024), (2048, 1024),
                              (3072, 896)]

                    def strip(ci, mm):
                        coff, cw = CHUNKS[ci]
                        base = (mm + 1) * P
                        ps = blk_pool.tile([P, 2, 512], F32, tag="blk")
                        flat = ps[:].rearrange("p a c -> p (a c)")
                        nsub = (cw + 511) // 512
                        for b in range(nsub):
                            w = min(512, cw - b * 512)
                            psv = ps[:, b, 0:w]
                            c0 = base + coff + b * 512
                            nc.tensor.matmul(
                                psv, lhsT=lhs(mm, 0),
                                rhs=ct[:, 0, c0:c0 + w],
                                start=True, stop=False)
                            nc.tensor.matmul(
                                psv, lhsT=lhs(mm, 1),
                                rhs=ct[:, 1, c0:c0 + w],
                                start=False, stop=False)
                            nc.tensor.matmul(
                                psv, lhsT=ones_r(),
                                rhs=sq2[:, c0:c0 + w],
                                start=False, stop=True)
                            wrot[0] += 1
                        tr = tr_pool.tile([P, 1024], BF16, tag="tr")
                        nc.scalar.activation(
                            tr[:, 0:cw], flat[:, 0:cw], Act.Sqrt,
                            bias=bias_sm[:, mm:mm + 1], scale=1.0,
                            accum_out=acc_w2[:, mm * 4 + ci:
                                             mm * 4 + ci + 1])

                    def combo(mm):
                        """diag (w=1) and d=32 (w=1) tiles, strided rhs."""
                        ps = blk_pool.tile([P, 2, 512], F32, tag="blk")
                        pd = ps[:, 0, 0:2 * P]
                        r0 = ct0v[:, mm:mm + 33:32, :]
                        r1 = ct1v[:, mm:mm + 33:32, :]
                        rs = sq2v[:, mm:mm + 33:32, :]
                        nc.tensor.matmul(pd, lhsT=lhs(mm, 0), rhs=r0,
                                         start=True, stop=False)
                        nc.tensor.matmul(pd, lhsT=lhs(mm, 1), rhs=r1,
                                         start=False, stop=False)
                        nc.tensor.matmul(pd, lhsT=ones_r(), rhs=rs,
                                         start=False, stop=True)
                        wrot[0] += 1
                        trd = trd_pool.tile([P, 2 * P], BF16, tag="trd")
                        nc.scalar.activation(
                            trd[:], pd, Act.Sqrt,
                            bias=bias_bg[:, mm:mm + 1], scale=1.0,
                            accum_out=acc_w1[:, mm:mm + 1])

                    # ---- interleaved emission: every engine's program is in
                    # data-arrival order so phase A (load/norm/transpose),
                    # sq2sum, and the distance blocks pipeline ----
                    bisectA = stage in (10, 11, 12, 13)
                    upto = {10: 0, 11: 1, 12: 2, 13: 3}.get(stage, 3)
                    full = stage >= 4 and not bisectA
                    has2 = stage >= 2 and not bisectA
                    has3 = stage >= 3 and not bisectA
                    if stage >= 1:
                        xb0, r0g = norms(0, 16)
                        if upto:
                            transposes(0, 16, xb0, r0g, upto)
                        if has3:
                            lhs_esum()
                        if has2:
                            sq2sum(0, 4)
                        xb1, r1g = norms(16, 16)
                        if full:
                            for m in range(3):
                                strip(0, m)
                        if upto:
                            transposes(16, 16, xb1, r1g, upto)
                        if has2:
                            sq2sum(4, 8)
                        xb2, r2g = norms(32, 8)
                        if full:
                            for m in range(3, LT):
                                strip(0, m)
                            for m in range(LT):
                                strip(1, m)
                        if upto:
                            transposes(32, 8, xb2, r2g, upto)
                        if has2:
                            sq2sum(8, 10)
                        if full:
                            for m in range(LT):
                                strip(2, m)
                            for m in range(LT):
                                strip(3, m)
                            for m in range(LT):
                                combo(m)

                    # ---- Phase E: final reductions and output ----
                    nc.vector.tensor_reduce(
                        out_sb[:, 0:1], s_loc[:], axis=AX.X, op=Op.add)
                    nc.vector.tensor_reduce(
                        out_sb[:, 1:2], acc_w1[:], axis=AX.X, op=Op.add)
                    nc.vector.tensor_reduce(
                        out_sb[:, 2:3], acc_w2[:], axis=AX.X, op=Op.add)
                    nc.vector.memset(out_sb[:, 3:4], 0.0)
                    nc.vector.tensor_copy(out_sb[:, 4:6], esum[:])
                    nc.vector.memset(out_sb[:, 6:8], 0.0)
                    nc.sync.dma_start(out=out, in_=out_sb[:])

    nc.compile()
    return nc


def kernel(embeddings: np.ndarray) -> np.ndarray:
    from concourse.bass_utils import run_bass_kernel_spmd

    X = np.ascontiguousarray(np.asarray(embeddings, dtype=np.float32))
    assert X.shape == (N, D)

    if "nc" not in _CACHE:
        _CACHE["nc"] = _build()
    nc = _CACHE["nc"]

    in_maps = [
        {"x": np.ascontiguousarray(np.roll(X, -k * LT * P, axis=0)[:NU])}
        for k in range(NCORES)
    ]
    res = run_bass_kernel_spmd(nc, in_maps, core_ids=list(range(NCORES)))

    s_sum = 0.0
    w1 = 0.0
    w2 = 0.0
    ecols = np.zeros(2 * P, dtype=np.float64)
    for k in range(NCORES):
        o = res.results[k]["out"]
        s_sum += float(o[:, 0].sum(dtype=np.float64))
        w1 += float(o[:, 1].sum(dtype=np.float64))
        w2 += float(o[:, 2].sum(dtype=np.float64))
        ecols += o[:, 4:6].astype(np.float64).T.reshape(-1)

    mu_sq = float(np.dot(ecols, ecols)) / (float(N) * float(N))
    dist_sum = w1 + 2.0 * w2
    mean_distance = dist_sum / (float(N) * float(N))
    diag_var_mean = (s_sum - float(N) * mu_sq) / float(D)
    loss = 1.0 / diag_var_mean + np.log(mean_distance)
    return np.float32(loss)
